# revision 72
# baseline (speedup 1.0000x reference)
"""Multi-head causal attention (B=2, T=2048, D=1024, H=16) on 8 trn2 NeuronCores.

Sharding: 8 cores = 2 batches x 4 head-groups (4 heads each). Each core:
  - computes qkv projections for its 4 heads from x[b] (host pre-shuffled to
    partition-major layout so every DMA line is contiguous),
  - runs masked softmax attention in transposed (k, q) score layout,
  - emits a partial output projection y_part = attn_heads @ w_out[head_rows]
    in bf16. Host sums the 4 partials per batch in f32.

All matmuls run in bf16 with fp32 PSUM accumulation. Softmax skips the
max-subtraction (scores are ~N(0,1)): both heads of a pair land in one
2-bank PSUM tile and a single ScalarE exp covers them; softmax denominators
come for free from an all-ones block appended to v (matmul cost depends
only on the moving dim). Mask handling is generic: the host classifies
(128k x 512q) blocks of the provided mask into skip / full / partial;
partial blocks get a narrowed exp plus a mask multiply on gpsimd.

Schedule: attention rows alternate between the two head-pairs; qkv/out
projections are interleaved into the exp-paced attention loop as PE filler
work with one-row lookahead flushes (so PSUM->SBUF casts clear the DVE
before a matmul needs them) and a tail reserve for the final-row drought.
PSUM accumulators are copied to SBUF in full at row end to release the
single-buffered psU banks immediately; y evictions run on ScalarE.
DMA traffic is split across the SP and ACT hardware queues (inputs in
need-order) plus per-half final-row y writes so the last write lands early.
"""
import sys
sys.path.insert(0, "/opt/trn_rl_repo")

import numpy as np
import ml_dtypes

import concourse.bass as bass
import concourse.mybir as mybir
import concourse.tile as tile
from concourse import bacc
from concourse.bass_utils import run_bass_kernel_spmd

B, T, D, H, Dh = 2, 2048, 1024, 16, 64
P = 128
QT = 512              # q-tile width (score tile free dim)
NQ = T // QT          # 4
NKT = T // P          # 16
ND = D // P           # 8
HPC = 4               # heads per core
NPAIR = HPC // 2      # head pairs per core
N_CORES = 8

f32 = mybir.dt.float32
bf16 = mybir.dt.bfloat16
CDT = bf16            # compute dtype for matmul operands
NP_CDT = ml_dtypes.bfloat16


def _block_structure(mask: np.ndarray):
    """Classify maskT (k,q) blocks: per q-tile a list of (kt, pattern_idx|None).

    For each unique partial pattern also derive (w0, m_lo, m_hi): w0 leading
    all-masked columns (exp skipped, memset 0), and [m_lo, m_hi) the column
    range that still needs the mask multiply.
    """
    maskT = (mask != 0).T.astype(np.float32)  # [k, q] visibility
    vis = []
    patterns = []
    meta = []
    pat_index = {}
    for qt in range(NQ):
        row = []
        for kt in range(NKT):
            blk = maskT[kt * P:(kt + 1) * P, qt * QT:(qt + 1) * QT]
            s = blk.sum()
            if s == 0:
                continue
            if s == blk.size:
                row.append((kt, None))
            else:
                key = blk.tobytes()
                if key not in pat_index:
                    pat_index[key] = len(patterns)
                    patterns.append(blk)
                    col_any = blk.any(axis=0)       # column has any visible
                    col_all = blk.all(axis=0)       # column fully visible
                    w0 = int(np.argmax(col_any)) if col_any.any() else QT
                    partial_cols = np.nonzero(col_any & ~col_all)[0]
                    if partial_cols.size:
                        m_lo, m_hi = int(partial_cols[0]), int(partial_cols[-1]) + 1
                    else:
                        m_lo = m_hi = 0
                    meta.append((w0, m_lo, m_hi))
                row.append((kt, pat_index[key]))
        vis.append(row)
    if patterns:
        pm = np.stack(patterns)
    else:
        pm = np.zeros((1, P, QT), np.float32)
    return vis, pm, meta


def _build_program(vis, n_pm, meta=(), compile=True):
    nc = bacc.Bacc() if compile else bass.Bass()
    # all inputs arrive host-pre-shuffled to partition-major layout so every
    # DMA line is contiguous (4KB for weights, 2KB for x chunks); x is stored
    # window-major [p, w, o, 512] so each (window, o-pair) chunk is contiguous
    xT = nc.declare_dram_parameter("xT", [P, NQ, ND, QT], CDT, isOutput=False)
    wq = nc.declare_dram_parameter("wq", [P, ND * HPC * Dh], CDT, isOutput=False)
    wk = nc.declare_dram_parameter("wk", [P, ND * HPC * Dh], CDT, isOutput=False)
    wv = nc.declare_dram_parameter("wv", [P, ND * HPC * Dh], CDT, isOutput=False)
    wo = nc.declare_dram_parameter("wo", [P, NPAIR * D], CDT, isOutput=False)
    pmask = nc.declare_dram_parameter("pmask", [P, n_pm * QT], CDT, isOutput=False)
    y = nc.declare_dram_parameter("y", [T, D], CDT, isOutput=True)

    inv_sqrt_dh = 1.0 / float(np.sqrt(Dh))

    with tile.TileContext(nc) as tc:
        with (
            tc.tile_pool(name="persist", bufs=1) as persist,
            tc.tile_pool(name="work", bufs=3) as work,
            tc.tile_pool(name="psA", bufs=2, space="PSUM") as psA,
            tc.tile_pool(name="psS", bufs=2, space="PSUM") as psS,
            tc.tile_pool(name="psU", bufs=1, space="PSUM") as psU,
        ):
            # ---- persistent SBUF tensors ----
            xt_sb = persist.tile([P, ND, T], CDT, tag="xt")
            wq_sb = persist.tile([P, ND, HPC * Dh], CDT, tag="wq")
            wk_sb = persist.tile([P, ND, HPC * Dh], CDT, tag="wk")
            wv_sb = persist.tile([P, ND, HPC * Dh], CDT, tag="wv")
            wo_sb = persist.tile([P, NPAIR, D], CDT, tag="wo")
            pm_sb = persist.tile([P, n_pm, QT], CDT, tag="pm")
            # per-pair tensors (separate tiles so cross-pair interleaving
            # cannot create false dependencies)
            qT_sb = [persist.tile([P, T], CDT, tag=f"qT{p}", name=f"qT{p}") for p in range(NPAIR)]
            kT_sb = [persist.tile([P, T], CDT, tag=f"kT{p}", name=f"kT{p}") for p in range(NPAIR)]
            at_sb = [persist.tile([P, T], CDT, tag=f"at{p}", name=f"at{p}") for p in range(NPAIR)]
            # v1: per k-tile and head, [128, 128]: for even heads cols 0:64 =
            # v values and cols 64:128 all-ones (for odd heads the reverse),
            # so the attnU matmul emits softmax denominators replicated on the
            # complementary partition half (matmul cost only depends on N).
            v1_sb = persist.tile([P, NKT, HPC, P], CDT, tag="v1")

            # ---- input DMAs split across both HWDGE queues (SP + ACT) ----
            # x loads per (window, o-pair) chunk: contiguous 2KB lines on the
            # DRAM side, with per-o dependency granularity so v/qk matmuls
            # start as each chunk lands. The first q-window goes first (it
            # gates the inline v tiles + first qk units); bulk triggers issue
            # from the idle SP sequencer so they never delay ACT's exp stream.
            def xo(w, o, eng):
                eng.dma_start(xt_sb[:, o:o + 2, w * QT:(w + 1) * QT],
                              xT[:, w, o:o + 2, :])

            xo(0, 0, nc.sync)
            nc.scalar.dma_start(wv_sb.rearrange("p o e -> p (o e)"), wv[:])
            xo(0, 2, nc.sync)
            xo(0, 4, nc.scalar)
            nc.sync.dma_start(wk_sb.rearrange("p o e -> p (o e)"), wk[:])
            xo(0, 6, nc.scalar)
            nc.scalar.dma_start(wq_sb.rearrange("p o e -> p (o e)"), wq[:])
            nc.sync.dma_start(pm_sb.rearrange("p n q -> p (n q)"), pmask[:])
            for w in range(1, NQ):
                for o in range(0, ND, 2):
                    xo(w, o, nc.sync)
            nc.sync.dma_start(wo_sb.rearrange("p o e -> p (o e)"), wo[:])

            # ---- filler queue: PE-side work interleaved into ACT-paced ----
            # ---- attention steps                                        ----
            fillers = []  # list of (key, thunk); emitted in order

            def drain(k, reserve=0):
                # keep `reserve` fillers back so the tail drought after the
                # final row's normalize still has PE work to chew on
                k = min(k, len(fillers) - reserve)
                for _ in range(max(k, 0)):
                    fillers.pop(0)[1]()

            def flush_through(pred):
                """Emit queued fillers (in order) until none matching pred remain."""
                while any(pred(key) for key, _ in fillers):
                    fillers.pop(0)[1]()

            # ---- phase A: v = x @ wv ----
            # ones blocks only (v halves are written by emit_v evictions):
            # even heads keep ones at cols Dh:P, odd heads at 0:Dh. On
            # gpsimd: keeps ~3.6us of memset off the DVE right when the
            # first v-tile evictions need it.
            nc.gpsimd.memset(v1_sb[:, :, 0::2, Dh:P], 1.0)
            nc.gpsimd.memset(v1_sb[:, :, 1::2, 0:Dh], 1.0)

            def emit_v(tt):
                ps_v = psA.tile([P, QT], f32, tag="psA", name=f"psv{tt}")
                for dt in range(ND):
                    nc.tensor.matmul(
                        ps_v[:, :HPC * Dh],
                        xt_sb[:, dt, tt * P:(tt + 1) * P],
                        wv_sb[:, dt, :],
                        start=(dt == 0),
                        stop=(dt == ND - 1),
                    )
                ps_vh = ps_v[:, :HPC * Dh].rearrange("p (h e) -> p h e", h=HPC)
                nc.vector.tensor_copy(v1_sb[:, tt, 0::2, 0:Dh], ps_vh[:, 0::2])
                nc.vector.tensor_copy(v1_sb[:, tt, 1::2, Dh:P], ps_vh[:, 1::2])

            # ---- phase A: qT / kT for pair p, one (tensor, nt) at a time ----
            def qk_units(p):
                units = []
                # nt-major, k before q: a flush of one row's qk then pops
                # exactly that row's units (k first: scores' stationary is
                # kT, and wk is loaded earlier)
                for nt in range(NQ):
                    for w_sb, out_sb in ((wk_sb, kT_sb[p]), (wq_sb, qT_sb[p])):
                        ps_box = []

                        def mm(dt, w_sb=w_sb, nt=nt, ps_box=ps_box, p=p):
                            if dt == 0:
                                ps_box.append(psA.tile(
                                    [P, QT], f32, tag="psA", name=f"psqk{p}_{nt}_{id(w_sb) % 97}"))
                            nc.tensor.matmul(
                                ps_box[0],
                                w_sb[:, dt, p * P:(p + 1) * P],
                                xt_sb[:, dt, nt * QT:(nt + 1) * QT],
                                start=(dt == 0),
                                stop=(dt == ND - 1),
                            )

                        def evict(out_sb=out_sb, nt=nt, ps_box=ps_box):
                            nc.vector.tensor_copy(
                                out_sb[:, nt * QT:(nt + 1) * QT], ps_box[0])

                        key = ("qk", p, nt)

                        def mk(dt, mm=mm):
                            return lambda: mm(dt)

                        units.extend((key, mk(dt)) for dt in range(ND))
                        units.append((key, evict))
                return units

            # ---- phase C: out-projection for one t-tile/half (as filler) ----
            # y partials leave as bf16 (halves output DMA traffic; host sums
            # the per-core partials in f32), one [128, D] DMA per t-tile.
            ysb_box = {}

            def make_outproj(tt, half):
                def go():
                    ps_y = psA.tile([P, QT], f32, tag="psA", name=f"psy{tt}_{half}")
                    for p in range(NPAIR):
                        nc.tensor.matmul(
                            ps_y[:],
                            at_sb[p][:, tt * P:(tt + 1) * P],
                            wo_sb[:, p, half * QT:(half + 1) * QT],
                            start=(p == 0),
                            stop=(p == NPAIR - 1),
                        )
                    if half == 0:
                        ysb_box[tt] = work.tile([P, D], CDT, tag="y", name=f"y{tt}")
                    ysb = ysb_box[tt]
                    # evict on ScalarE (activation Copy): keeps the sink-only
                    # y traffic out of the DVE stream, whose row-end backlog
                    # gates the normalize chain and the psA ring. The tts
                    # that drain in the EPILOGUE (after the final normalize
                    # is already emitted) go on DVE instead: their psA slots
                    # then free at DVE pace instead of queuing behind ACT's
                    # exp backlog; the final row alternates DVE/ACT.
                    if (T // P - 2 * (QT // P)) <= tt < (T // P - QT // P):
                        on_dve = True
                    elif tt >= T // P - QT // P:
                        on_dve = half == 0
                    else:
                        on_dve = False
                    if on_dve:
                        nc.vector.tensor_copy(
                            ysb[:, half * QT:(half + 1) * QT], ps_y[:])
                    else:
                        nc.scalar.activation(
                            ysb[:, half * QT:(half + 1) * QT], ps_y[:],
                            mybir.ActivationFunctionType.Copy)
                    if tt >= T // P - QT // P:
                        # final row: per-half DMA, bytes split across both HW
                        # queues; the very last tile issues from ACT (same
                        # engine as the eviction -- no cross-engine semaphore)
                        eng = nc.scalar if tt == T // P - 1 else nc.sync
                        eng.dma_start(
                            y[tt * P:(tt + 1) * P, half * QT:(half + 1) * QT],
                            ysb[:, half * QT:(half + 1) * QT])
                        if half == 1:
                            ysb_box.pop(tt)
                    elif half == 1:
                        nc.sync.dma_start(y[tt * P:(tt + 1) * P, :], ysb_box.pop(tt)[:])
                return go

            # ---- phase B: attention rows (pair p, q-tile qt), software-
            # ---- pipelined; rows alternate between pairs so filler supply
            # ---- (qk of the other pair, v tail, out-projections) spreads
            # ---- evenly instead of running dry during the second pair.
            # ---- The scores->exp pipeline runs CONTINUOUSLY across row
            # ---- boundaries: the last two iterations of row r emit row
            # ---- r+1's first two score blocks, so ACT never drains dry and
            # ---- the new row's scores don't stall on a psS ring slot.
            rows = [(p, qt) for qt in range(NQ) for p in range(NPAIR)]
            row_es = {}  # r -> [(es_tile, w0), ...] in block order

            def emit_scores_for(r, j):
                p, qt = rows[r]
                row = vis[qt]
                kt, pidx = row[j]
                w0 = 0 if pidx is None else meta[pidx][0]
                if j == 0:
                    w0 = 0  # first accumulation must set has_written
                # both heads' scores land in one 2-bank PSUM tile so a
                # single exp instruction covers them (halves ACT
                # instruction + semaphore overhead on the pacing chain)
                ps_s = psS.tile([P, 2, QT], f32, tag="s", name=f"s_{p}_{qt}_{kt}")
                for h in range(2):
                    base = h * Dh
                    nc.tensor.matmul(
                        ps_s[:, h, w0:QT],
                        kT_sb[p][base:base + Dh, kt * P:(kt + 1) * P],
                        qT_sb[p][base:base + Dh, qt * QT + w0:(qt + 1) * QT],
                        start=True,
                        stop=True,
                        tile_position=(base, 0),
                    )
                es = work.tile([P, 2, QT], CDT, tag="es", name=f"es_{p}_{qt}_{kt}")
                nc.scalar.activation(
                    es[:, :, w0:QT], ps_s[:, :, w0:QT],
                    mybir.ActivationFunctionType.Exp,
                    scale=inv_sqrt_dh,
                )
                if pidx is not None:
                    _w0, m_lo, m_hi = meta[pidx]
                    m_lo = min(m_lo, w0)  # w0 forced to 0 on j==0
                    if m_hi > m_lo:
                        # gpsimd, not DVE: the diagonal blocks sit at row
                        # ends where the DVE is busy with the normalize +
                        # eviction burst; Pool is idle
                        for h in range(2):
                            nc.gpsimd.tensor_mul(
                                es[:, h, m_lo:m_hi], es[:, h, m_lo:m_hi],
                                pm_sb[:, pidx, m_lo:m_hi],
                            )
                row_es.setdefault(r, []).append((es, w0))

            def spill_scores(r, j2):
                """Emit score block j2 of row r if it exists and is next."""
                if r >= len(rows):
                    return
                row2 = vis[rows[r][1]]
                if j2 < min(2, len(row2)) and len(row_es.setdefault(r, [])) == j2:
                    emit_scores_for(r, j2)

            def emit_attention_row(r, nxt=None, nxt_at_j1=False):
                    p, qt = rows[r]
                    row = vis[qt]
                    max_kt = max((kt for kt, _ in row), default=-1)
                    # flush this row's deps, plus the NEXT row's qk and v
                    # units so their PSUM->SBUF casts clear the DVE a full row
                    # before an attnU/scores matmul needs the result. On the
                    # first row the next row's qk flush is deferred into the
                    # block loop (j==1) instead: still ahead of the cross-row
                    # score spill, but not ahead of this row's first scores.
                    nxt_kt = -1 if nxt is None else max(
                        (kt for kt, _ in vis[nxt[1]]), default=-1)
                    flush_through(lambda key: (
                        (key[0] == "v" and key[1] <= max(max_kt, nxt_kt))
                        or (key[0] == "qk" and (key[1], key[2]) == (p, qt))
                        or (nxt is not None and not nxt_at_j1
                            and key[0] == "qk" and (key[1], key[2]) == nxt)))
                    if not row:
                        for h in range(2):
                            nc.vector.memset(
                                at_sb[p][h * Dh:(h + 1) * Dh, qt * QT:(qt + 1) * QT], 0.0)
                        return
                    ps_u = [
                        psU.tile([P, QT], f32, tag=f"u{h}", name=f"u{h}_{p}_{qt}")
                        for h in range(2)
                    ]
                    es_q = row_es.setdefault(r, [])

                    def emit_attnu(j):
                        kt, _ = row[j]
                        es, w0 = es_q[j]
                        for h in range(2):
                            nc.tensor.matmul(
                                ps_u[h][:, w0:QT],
                                v1_sb[:, kt, 2 * p + h, :],
                                es[:, h, w0:QT],
                                start=(j == 0),
                                stop=(j == len(row) - 1),
                            )

                    # hold back ready fillers near the end so the tail drought
                    # (final normalize -> outproj chain) has PE work; release
                    # half during the final (filler-starved, ACT-paced) row
                    rsv = 8 if r >= len(rows) - 3 else 0
                    drain(2, rsv)
                    while len(es_q) < min(2, len(row)):
                        emit_scores_for(r, len(es_q))
                    nxt_flush_j = max(0, min(1, len(row) - 2))
                    for j in range(len(row)):
                        if nxt_at_j1 and nxt is not None and j == nxt_flush_j:
                            flush_through(lambda key: (
                                key[0] == "qk" and (key[1], key[2]) == nxt))
                        drain(2, rsv)
                        nj = j + 2
                        if nj < len(row):
                            emit_scores_for(r, nj)
                        elif nxt is not None:
                            # spill only into a row whose qk units were
                            # flushed above -- otherwise the spilled scores
                            # would read qT/kT before they are written
                            spill_scores(r + 1, nj - len(row))
                        emit_attnu(j)

                    # ps_u[0]: partitions 0:64 = attnU, 64:128 = denominators
                    # (ones half of v1); ps_u[1] the reverse. Copy both PSUM
                    # accumulators to SBUF in full right away: that releases
                    # the single-buffered psU banks so the next row's attnU
                    # can start, and the rest of the normalize chain (recip on
                    # DVE at base partition 0, cross-half moves by DMA) runs
                    # from SBUF off the critical path.
                    u_sb = []
                    for h in range(2):
                        u = work.tile([P, QT], f32, tag=f"usb{h}", name=f"u_sb{h}_{p}_{qt}")
                        nc.vector.tensor_copy(u[:], ps_u[h][:])
                        u_sb.append(u)
                    sums2 = work.tile([P, QT], f32, tag="sums2", name=f"s2_{p}_{qt}")
                    nc.sync.dma_start(sums2[0:Dh, :], u_sb[0][Dh:P, :])
                    rep1 = work.tile([P, QT], f32, tag="rep1", name=f"rp1_{p}_{qt}")
                    nc.vector.reciprocal_approx_fast(rep1[0:Dh, :], u_sb[1][0:Dh, :])
                    rep2 = work.tile([P, QT], f32, tag="rep2", name=f"rp2_{p}_{qt}")
                    nc.sync.dma_start(rep2[Dh:P, :], rep1[0:Dh, :])
                    rep0 = work.tile([P, QT], f32, tag="rep0", name=f"rp0_{p}_{qt}")
                    nc.vector.reciprocal_approx_fast(rep0[0:Dh, :], sums2[0:Dh, :])
                    nc.vector.tensor_mul(
                        at_sb[p][0:Dh, qt * QT:(qt + 1) * QT],
                        u_sb[0][0:Dh, :],
                        rep0[0:Dh, :],
                    )
                    nc.vector.tensor_mul(
                        at_sb[p][Dh:P, qt * QT:(qt + 1) * QT],
                        u_sb[1][Dh:P, :],
                        rep2[Dh:P, :],
                    )
                    if p == NPAIR - 1:
                        # out-projection for the t-tiles this qt completed
                        fillers.extend(
                            (("op", tt, half), make_outproj(tt, half))
                            for tt in range(qt * (QT // P), (qt + 1) * (QT // P))
                            for half in range(2)
                        )

            # inline prologue: just enough for attention(p0, qt0) to start
            for tt in range(NQ):
                emit_v(tt)
            fillers.extend((("v", tt), (lambda tt=tt: emit_v(tt))) for tt in range(NQ, NKT))
            fillers.extend(qk_units(0))
            fillers.extend(qk_units(1))
            for r in range(len(rows)):
                nxt = rows[r + 1] if r + 1 < len(rows) else None
                # first row: defer the next row's qk flush into the block
                # loop so it doesn't push this row's first scores (and the
                # ACT exp pipeline start) several microseconds later
                emit_attention_row(r, nxt, nxt_at_j1=(r == 0))
            while fillers:
                drain(len(fillers))
    if compile:
        nc.compile()
    return nc


def _shuf_rows(w, groups):
    """[groups*128, C] -> [128, groups*C] partition-major (contiguous lines)."""
    c = w.shape[1]
    return np.ascontiguousarray(
        w.reshape(groups, P, c).transpose(1, 0, 2).reshape(P, groups * c))


def _host_inputs(x, mask, w_qkv, w_out):
    vis, pm, meta = _block_structure(np.asarray(mask))
    pm_c = np.ascontiguousarray(pm.astype(NP_CDT).transpose(1, 0, 2).reshape(P, -1))
    wq_f, wk_f, wv_f = np.split(np.asarray(w_qkv, np.float32), 3, axis=1)
    in_maps = []
    for core in range(N_CORES):
        b = core // 4
        g = core % 4
        cols = slice(g * HPC * Dh, (g + 1) * HPC * Dh)
        in_maps.append({
            "xT": np.ascontiguousarray(
                _shuf_rows(
                    np.ascontiguousarray(np.asarray(x[b], np.float32).T)
                    .astype(NP_CDT), ND)
                .reshape(P, ND, NQ, QT).transpose(0, 2, 1, 3)),
            "wq": _shuf_rows(wq_f[:, cols].astype(NP_CDT), ND),
            "wk": _shuf_rows(wk_f[:, cols].astype(NP_CDT), ND),
            "wv": _shuf_rows(wv_f[:, cols].astype(NP_CDT), ND),
            "wo": _shuf_rows(np.asarray(w_out, np.float32)[cols, :].astype(NP_CDT),
                             NPAIR),
            "pmask": pm_c,
        })
    return vis, pm, meta, in_maps


def run(x, mask, w_qkv, w_out, trace=False):
    import os
    vis, pm, meta, in_maps = _host_inputs(x, mask, w_qkv, w_out)
    nc = _build_program(vis, pm.shape[0], meta)
    if not trace:
        # An inherited BASS_TRACE=1 would pull in NTFF profiling hooks that
        # may not exist in this environment; force tracing off.
        os.environ["BASS_NEVER_TRACE"] = "1"
    else:
        os.environ.pop("BASS_NEVER_TRACE", None)
    res = run_bass_kernel_spmd(nc, in_maps, core_ids=list(range(N_CORES)), trace=trace)
    parts = [res.results[i]["y"].astype(np.float32) for i in range(N_CORES)]
    out = np.stack([
        parts[0] + parts[1] + parts[2] + parts[3],
        parts[4] + parts[5] + parts[6] + parts[7],
    ]).astype(np.float32)
    return out, res


def kernel(x, mask, w_qkv, w_out):
    out, _ = run(x, mask, w_qkv, w_out, trace=False)
    return out

